# revision 5
# baseline (speedup 1.0000x reference)
"""3x3 conv (im2col matmul): x[16,128,56,56] * w[256,128,3,3] + b -> [16,256,56,56].

Runs on 8 TRN2 NeuronCores, data-parallel over the batch dim (2 images/core).
Per core: 9 accumulating f32r matmuls per PSUM tile (K=C=128, N=448 = 8 output
rows), 2 Cout halves x 7 row-tiles x 2 images; ScalarE activation(Copy, bias)
as the fused PSUM->SBUF epilogue; 4 large output DMAs.
"""
import numpy as np

N_CORES = 8
C, COUT, H, W = 128, 256, 56, 56
HP, WP = H + 2, W + 2          # 58, 58 (host-padded)
LP = HP * WP                   # 3364
L = H * W                      # 3136
RT = 7                         # row tiles per image (8 rows each)
NT = 8 * W                     # 448 columns per matmul

_RUNNER = None


def _build_nc(repeat=1):
    import concourse.tile as tile
    from concourse import bacc, mybir

    F32 = mybir.dt.float32
    F32R = mybir.dt.float32r
    ACT_IDENT = mybir.ActivationFunctionType.Identity

    nc = bacc.Bacc("TRN2", target_bir_lowering=False, debug=False)
    xp = nc.dram_tensor("xp", [2, C, LP], F32R, kind="ExternalInput")
    wt = nc.dram_tensor("wt", [C, 9 * COUT], F32R, kind="ExternalInput")
    bias = nc.dram_tensor("bias", [2, 128], F32, kind="ExternalInput")
    out = nc.dram_tensor("out", [2, COUT, L], F32, kind="ExternalOutput")

    with tile.TileContext(nc, trace_sim=False) as tc:
        with (
            tc.tile_pool(name="const", bufs=1) as cpool,
            tc.tile_pool(name="xin", bufs=2) as xpool,
            tc.tile_pool(name="ob", bufs=2) as opool,
            tc.tile_pool(name="ps", bufs=8, space="PSUM") as ps,
        ):
            wt_sb = cpool.tile([C, 9 * COUT], F32R, tag="wt")
            bias_sb = cpool.tile([128, 2], F32, tag="bias")
            nc.sync.dma_start(wt_sb[:], wt[:])
            for h in range(2):
                nc.sync.dma_start(bias_sb[:, h:h + 1],
                                  bias[h].rearrange("(p o) -> p o", o=1))
            for _rep in range(repeat):
                x_sb = []
                for n in range(2):
                    t = xpool.tile([C, LP], F32R, tag=f"x{n}")
                    nc.sync.dma_start(t[:], xp[n])
                    x_sb.append(t[:].rearrange("p (r w) -> p r w", w=WP))
                for n in range(2):
                    for h in range(2):
                        osb = opool.tile([128, L], F32, tag="osb")
                        for rt in range(RT):
                            acc = ps.tile([128, NT], F32, tag="acc")
                            for r in range(9):
                                di, dj = divmod(r, 3)
                                rhs = x_sb[n][:, rt * 8 + di: rt * 8 + di + 8,
                                              dj: dj + W]
                                base = r * COUT + h * 128
                                nc.tensor.matmul(
                                    acc[:], wt_sb[:, base:base + 128], rhs,
                                    start=(r == 0), stop=(r == 8))
                            nc.scalar.activation(
                                osb[:, rt * NT:(rt + 1) * NT], acc[:],
                                ACT_IDENT, bias=bias_sb[:, h:h + 1])
                        nc.sync.dma_start(out[n, h * 128:(h + 1) * 128, :],
                                          osb[:])
    nc.compile()
    return nc


def _make_runner(nc):
    import jax
    from jax.experimental.shard_map import shard_map
    from jax.sharding import Mesh, PartitionSpec
    from concourse import mybir
    from concourse.bass2jax import (_bass_exec_p, install_neuronx_cc_hook,
                                    partition_id_tensor)

    install_neuronx_cc_hook()
    partition_name = (nc.partition_id_tensor.name
                      if nc.partition_id_tensor else None)
    in_names, out_names, out_avals, zero_shapes = [], [], [], []
    for alloc in nc.m.functions[0].allocations:
        if not isinstance(alloc, mybir.MemoryLocationSet):
            continue
        name = alloc.memorylocations[0].name
        if alloc.kind == "ExternalInput":
            if name != partition_name:
                in_names.append(name)
        elif alloc.kind == "ExternalOutput":
            out_names.append(name)
            shape = tuple(alloc.tensor_shape)
            dtype = mybir.dt.np(alloc.dtype)
            out_avals.append(jax.core.ShapedArray(shape, dtype))
            zero_shapes.append((shape, dtype))
    n_params, n_outs = len(in_names), len(out_names)
    donate = tuple(range(n_params, n_params + n_outs))

    def _body(*args):
        operands = list(args)
        all_names = in_names + out_names
        if partition_name is not None:
            operands.append(partition_id_tensor())
            all_names = all_names + [partition_name]
        outs = _bass_exec_p.bind(
            *operands,
            out_avals=tuple(out_avals),
            in_names=tuple(all_names),
            out_names=tuple(out_names),
            lowering_input_output_aliases=(),
            sim_require_finite=True,
            sim_require_nnan=True,
            nc=nc,
        )
        return tuple(outs)

    devices = jax.devices()[:N_CORES]
    mesh = Mesh(np.asarray(devices), ("core",))
    sharded = jax.jit(
        shard_map(_body, mesh=mesh,
                  in_specs=(PartitionSpec("core"),) * (n_params + n_outs),
                  out_specs=(PartitionSpec("core"),) * n_outs,
                  check_rep=False),
        donate_argnums=donate, keep_unused=True)

    def run(in_maps):
        concat_in = [np.concatenate([m[k] for m in in_maps], axis=0)
                     for k in in_names]
        concat_zeros = [np.zeros((N_CORES * s[0], *s[1:]), dt)
                        for s, dt in zero_shapes]
        out_arrs = sharded(*concat_in, *concat_zeros)
        return {name: np.asarray(out_arrs[i]).reshape(
                    N_CORES, *zero_shapes[i][0])
                for i, name in enumerate(out_names)}

    return run


def _prep_inputs(x, weight, bias):
    x = np.ascontiguousarray(x, dtype=np.float32)
    xpad = np.zeros((16, C, HP, WP), np.float32)
    xpad[:, :, 1:1 + H, 1:1 + W] = x
    xpad = xpad.reshape(16, C, LP)
    wt = np.ascontiguousarray(
        weight.astype(np.float32).transpose(1, 2, 3, 0).reshape(C, 9 * COUT))
    b2 = np.ascontiguousarray(bias.astype(np.float32).reshape(2, 128))
    return xpad, wt, b2


def kernel(x, weight, bias):
    global _RUNNER
    xpad, wt, b2 = _prep_inputs(x, weight, bias)
    if _RUNNER is None:
        _RUNNER = _make_runner(_build_nc(1))
    in_maps = [{"xp": xpad[2 * c:2 * c + 2], "wt": wt, "bias": b2}
               for c in range(N_CORES)]
    outs = _RUNNER(in_maps)
    y = outs["out"]  # [8, 2, 256, 3136]
    return y.reshape(16, COUT, H, W)


# revision 6
# speedup vs baseline: 74.2702x; 74.2702x over previous
"""3x3 conv (im2col matmul): x[16,128,56,56] * w[256,128,3,3] + b -> [16,256,56,56].

Runs on 8 TRN2 NeuronCores, data-parallel over the batch dim (2 images/core).
Per core: 9 accumulating matmuls per PSUM tile (K=C=128, N=448 = 8 output
rows), 2 Cout halves x 7 row-tiles x 2 images; ScalarE activation(Identity,
bias) as the fused PSUM->SBUF epilogue; 4 large output DMAs.
"""
import numpy as np

N_CORES = 8
C, COUT, H, W = 128, 256, 56, 56
HP, WP = H + 2, W + 2          # 58, 58 (host-padded)
LP = HP * WP                   # 3364
L = H * W                      # 3136
RT = 7                         # row tiles per image (8 rows each)
NT = 8 * W                     # 448 columns per matmul

DTYPE = "f32r"                 # matmul input dtype: "f32r" | "bf16" | "f32"

_RUNNER = None


def _np_in_dtype(dtype):
    if dtype == "bf16":
        import ml_dtypes
        return ml_dtypes.bfloat16
    return np.float32


def _build_nc(repeat=1, dtype=DTYPE, loop=False):
    import concourse.tile as tile
    from concourse import bacc, mybir

    F32 = mybir.dt.float32
    in_dt = {"f32r": mybir.dt.float32r, "bf16": mybir.dt.bfloat16,
             "f32": F32}[dtype]
    ACT_IDENT = mybir.ActivationFunctionType.Identity

    nc = bacc.Bacc("TRN2", target_bir_lowering=False, debug=False)
    xp = nc.dram_tensor("xp", [2, C, LP], in_dt, kind="ExternalInput")
    wt = nc.dram_tensor("wt", [C, 9 * COUT], in_dt, kind="ExternalInput")
    bias = nc.dram_tensor("bias", [2, 128], F32, kind="ExternalInput")
    out = nc.dram_tensor("out", [2, COUT, L], F32, kind="ExternalOutput")

    with tile.TileContext(nc, trace_sim=False) as tc:
        with (
            tc.tile_pool(name="const", bufs=1) as cpool,
            tc.tile_pool(name="xin", bufs=2) as xpool,
            tc.tile_pool(name="ob", bufs=4) as opool,
            tc.tile_pool(name="ps", bufs=8, space="PSUM") as ps,
        ):
            wt_sb = cpool.tile([C, 9 * COUT], in_dt, tag="wt")
            bias_sb = cpool.tile([128, 2], F32, tag="bias")

            def body():
                x_sb = []
                for n in range(2):
                    t = xpool.tile([C, LP], in_dt, tag=f"x{n}")
                    nc.sync.dma_start(t[:], xp[n])
                    x_sb.append(t[:].rearrange("p (r w) -> p r w", w=WP))
                    if n == 0:
                        nc.sync.dma_start(wt_sb[:], wt[:])
                        for h in range(2):
                            nc.sync.dma_start(
                                bias_sb[:, h:h + 1],
                                bias[h].rearrange("(p o) -> p o", o=1))
                for n in range(2):
                    for h in range(2):
                        osb = opool.tile([128, L], F32, tag="osb")
                        for rt in range(RT):
                            acc = ps.tile([128, NT], F32, tag="acc")
                            for r in range(9):
                                di, dj = divmod(r, 3)
                                rhs = x_sb[n][:, rt * 8 + di: rt * 8 + di + 8,
                                              dj: dj + W]
                                base = r * COUT + h * 128
                                nc.tensor.matmul(
                                    acc[:], wt_sb[:, base:base + 128], rhs,
                                    start=(r == 0), stop=(r == 8))
                            nc.scalar.activation(
                                osb[:, rt * NT:(rt + 1) * NT], acc[:],
                                ACT_IDENT, bias=bias_sb[:, h:h + 1])
                        nc.sync.dma_start(out[n, h * 128:(h + 1) * 128, :],
                                          osb[:])

            if loop and repeat > 1:
                hint = (mybir.EngineType.PE, mybir.EngineType.Activation,
                        mybir.EngineType.SP)
                with tc.For_i(0, repeat, 1, hint_engines=hint):
                    body()
            else:
                for _ in range(repeat):
                    body()
    nc.compile()
    return nc


def _make_runner(nc, donate=True):
    import jax
    from jax.experimental.shard_map import shard_map
    from jax.sharding import Mesh, PartitionSpec
    from concourse import mybir
    from concourse.bass2jax import (_bass_exec_p, install_neuronx_cc_hook,
                                    partition_id_tensor)

    install_neuronx_cc_hook()
    partition_name = (nc.partition_id_tensor.name
                      if nc.partition_id_tensor else None)
    in_names, out_names, out_avals, zero_shapes = [], [], [], []
    for alloc in nc.m.functions[0].allocations:
        if not isinstance(alloc, mybir.MemoryLocationSet):
            continue
        name = alloc.memorylocations[0].name
        if alloc.kind == "ExternalInput":
            if name != partition_name:
                in_names.append(name)
        elif alloc.kind == "ExternalOutput":
            out_names.append(name)
            shape = tuple(alloc.tensor_shape)
            dtype = mybir.dt.np(alloc.dtype)
            out_avals.append(jax.core.ShapedArray(shape, dtype))
            zero_shapes.append((shape, dtype))
    n_params, n_outs = len(in_names), len(out_names)
    donate_nums = tuple(range(n_params, n_params + n_outs)) if donate else ()

    def _body(*args):
        operands = list(args)
        all_names = in_names + out_names
        if partition_name is not None:
            operands.append(partition_id_tensor())
            all_names = all_names + [partition_name]
        outs = _bass_exec_p.bind(
            *operands,
            out_avals=tuple(out_avals),
            in_names=tuple(all_names),
            out_names=tuple(out_names),
            lowering_input_output_aliases=(),
            sim_require_finite=True,
            sim_require_nnan=True,
            nc=nc,
        )
        return tuple(outs)

    devices = jax.devices()[:N_CORES]
    mesh = Mesh(np.asarray(devices), ("core",))
    sharded = jax.jit(
        shard_map(_body, mesh=mesh,
                  in_specs=(PartitionSpec("core"),) * (n_params + n_outs),
                  out_specs=(PartitionSpec("core"),) * n_outs,
                  check_rep=False),
        donate_argnums=donate_nums, keep_unused=True)

    def run(in_maps):
        concat_in = [np.concatenate([m[k] for m in in_maps], axis=0)
                     for k in in_names]
        concat_zeros = [np.zeros((N_CORES * s[0], *s[1:]), dt)
                        for s, dt in zero_shapes]
        out_arrs = sharded(*concat_in, *concat_zeros)
        return {name: np.asarray(out_arrs[i]).reshape(
                    N_CORES, *zero_shapes[i][0])
                for i, name in enumerate(out_names)}

    run.sharded = sharded
    run.in_names = in_names
    run.zero_shapes = zero_shapes
    return run


def _prep_inputs(x, weight, bias, dtype=DTYPE):
    np_dt = _np_in_dtype(dtype)
    x = np.ascontiguousarray(x, dtype=np.float32)
    xpad = np.zeros((16, C, HP, WP), np.float32)
    xpad[:, :, 1:1 + H, 1:1 + W] = x
    xpad = np.ascontiguousarray(xpad.reshape(16, C, LP).astype(np_dt))
    wt = np.ascontiguousarray(
        weight.astype(np.float32).transpose(1, 2, 3, 0)
        .reshape(C, 9 * COUT).astype(np_dt))
    b2 = np.ascontiguousarray(bias.astype(np.float32).reshape(2, 128))
    return xpad, wt, b2


def kernel(x, weight, bias):
    global _RUNNER
    xpad, wt, b2 = _prep_inputs(x, weight, bias)
    if _RUNNER is None:
        _RUNNER = _make_runner(_build_nc(1))
    in_maps = [{"xp": xpad[2 * c:2 * c + 2], "wt": wt, "bias": b2}
               for c in range(N_CORES)]
    outs = _RUNNER(in_maps)
    y = outs["out"]  # [8, 2, 256, 3136]
    return y.reshape(16, COUT, H, W)
